# revision 37
# baseline (speedup 1.0000x reference)
"""GRU cell (AnotherGRUCell) on 8 TRN2 NeuronCores.

Strategy: pure data-parallel over batch (8192 rows -> 1024 rows/core),
weights replicated. No collectives.

All on-chip compute is in TRANSPOSED layout (units on the partition
axis, batch on the free axis), as in the bf16 baseline, but with a
mixed fp8/fp16 precision scheme chosen from a host-side error model
(validated against HW to 4 digits on the bf16 version):

  - matmul halves run either as fp8e4 (e4m3) DoubleRow matmuls (2
    contraction rows per PE cell per cycle -> ~2x bf16 throughput) or
    as fp16 matmuls (same speed as bf16 but 10-bit mantissa, which
    removes nearly all of the bf16 quantization error).
  - error budget (rel tol 2e-2): per-half err^2 contributions measured
    on the real inputs: r-gate halves ~0.15e-4 each, u-gate halves
    ~1.63e-4, cand-x 4.04e-4, cand-rh 1.29e-4. Chosen config: r fully
    fp8, cand rh-half fp8, u x-half fully fp8 (N8_UX=8 pairs),
    everything else fp16 -> rel err 1.792e-2 (sim == HW to 6 digits).
  - all weights (both dtypes) are pre-scaled by S=32 so fp8 weights
    avoid denormals; the sigmoid/tanh activation reads PSUM with
    scale=1/S, folding the rescale into the existing ScalarE op.

Layouts: fp8 moving operands are packed as [128, 2, B] pair tiles
(DoubleRow wants 3D APs [K=128, Ko=2, N] over two adjacent 128-row
k-subtiles); fp8 weights as [128, 2*len8, 128] col-pair slabs; fp16
weights as [128, len16, 128] per-col-tile slabs.
"""

import numpy as np
import ml_dtypes

import concourse.bacc as bacc
import concourse.tile as tile
import concourse.mybir as mybir
from concourse.bass_utils import run_bass_kernel_spmd

N_CORES = 8
UNITS = 2048
IN_DIM = 2048
BATCH = 8192
B_LOC = BATCH // N_CORES  # 1024 batch rows per core

P = 128
KSUB = 32            # 32 contraction k-subtiles for [x; h] (16 x + 16 h)
NT = UNITS // P      # 16 col-tiles per gate (r / u / cand)
M_CHUNK = 512
MC = B_LOC // M_CHUNK  # 2 moving chunks per core

S = 32.0             # weight pre-scale (fp8 denormal avoidance)

# fp8 coverage (in DoubleRow k-pairs, each pair = 2 k-subtiles of 128):
# r: both halves fully fp8; u: N8_UX pairs of the x-half; c: rh-half.
N8_UX = 8
LEN8_U = 2 * N8_UX          # fp8 k-subtiles per u col-tile
LEN16_U = (16 - LEN8_U) + 16  # fp16 k-subtiles per u col-tile

F8 = mybir.dt.float8e4
F16 = mybir.dt.float16
F32 = mybir.dt.float32
BF16 = mybir.dt.bfloat16
NP_F8 = ml_dtypes.float8_e4m3
NP_F16 = np.float16
DR = mybir.MatmulPerfMode.DoubleRow

_CACHED_NC = None

# test.py sets TRACE=True to capture the NTFF profile (exec_time_ns +
# perfetto trace); the graded path leaves it off. LAST_RESULTS holds the
# BassKernelResults of the most recent run.
TRACE = False
LAST_RESULTS = None


def _build():
    nc = bacc.Bacc("TRN2", target_bir_lowering=False, debug=False)

    x8d = nc.dram_tensor("x8d", [8, P, 2, B_LOC], F8, kind="ExternalInput")
    h8d = nc.dram_tensor("h8d", [8, P, 2, B_LOC], F8, kind="ExternalInput")
    # fp16 moving tensors in groups of 4 k-subtiles (fewer tiles -> fewer
    # PE first-use semaphore beats)
    x16d = nc.dram_tensor("x16d", [4, P, 4, B_LOC], F16, kind="ExternalInput")
    h16d = nc.dram_tensor("h16d", [4, P, 4, B_LOC], F16, kind="ExternalInput")
    # weights: fp8 slabs per col-PAIR [pair, 128, 2*len8, 128]; fp16
    # slabs per col-tile [t, 128, len16, 128]
    wr8 = nc.dram_tensor("wr8", [8, P, 2 * KSUB, P], F8, kind="ExternalInput")
    wu8 = nc.dram_tensor("wu8", [8, P, 2 * LEN8_U, P], F8, kind="ExternalInput")
    wu16 = nc.dram_tensor("wu16", [8, P, 2 * LEN16_U, P], F16,
                          kind="ExternalInput")
    wc16 = nc.dram_tensor("wc16", [8, P, 2 * 16, P], F16, kind="ExternalInput")
    wc8 = nc.dram_tensor("wc8", [8, P, 2 * 16, P], F8, kind="ExternalInput")
    brd = nc.dram_tensor("brd", [P, NT], F32, kind="ExternalInput")
    bud = nc.dram_tensor("bud", [P, NT], F32, kind="ExternalInput")
    bcd = nc.dram_tensor("bcd", [P, NT], F32, kind="ExternalInput")
    out = nc.dram_tensor("out", [NT, P, B_LOC], F16, kind="ExternalOutput")

    SIG = mybir.ActivationFunctionType.Sigmoid
    TANH = mybir.ActivationFunctionType.Tanh
    INV_S = 1.0 / S

    with tile.TileContext(nc) as tc:
        with (
            tc.tile_pool(name="resident", bufs=1) as res,
            tc.tile_pool(name="wslab", bufs=2) as wp,
            tc.tile_pool(name="psum", bufs=8, space="PSUM") as pp,
            tc.tile_pool(name="stage", bufs=2) as sp,
            tc.tile_pool(name="bias", bufs=1) as bp,
        ):
            x8t = [res.tile([P, 2, B_LOC], F8, tag=f"x8{j}", name=f"x8{j}")
                   for j in range(8)]
            h8t = [res.tile([P, 2, B_LOC], F8, tag=f"h8{j}", name=f"h8{j}")
                   for j in range(8)]
            x16g = [res.tile([P, 4, B_LOC], F16, tag=f"x16g{g}",
                             name=f"x16g{g}") for g in range(4)]
            h16g = [res.tile([P, 4, B_LOC], F16, tag=f"h16g{g}",
                             name=f"h16g{g}") for g in range(4)]
            rh8t = [res.tile([P, 2, B_LOC], F8, tag=f"rh{j}", name=f"rh{j}")
                    for j in range(8)]
            # u16 tiles are all aliased onto buffers that go dead before the
            # u phase: [0..7] onto the largest startup weight-chunk tiles,
            # [8..15] onto the h8 pair tiles (same 2KB/partition footprint);
            # the tile framework sequences the reuse. Allocated lazily in
            # the u loop.
            u16t = [None] * 16

            def x16_ap(j, ms):
                return x16g[j // 4][:, j % 4, ms]

            def h16_ap(j, ms):
                return h16g[j // 4][:, j % 4, ms]

            # PE warm-up vs the HAM clock gate (see bf16 baseline notes):
            # 8 dummy matmuls sized to end before real operands land.
            warm_src = sp.tile([P, M_CHUNK], BF16, tag="rtmp",
                               name="warm_src")
            nc.gpsimd.memset(warm_src[:], 0.0)
            warm_ps = pp.tile([P, M_CHUNK], F32, tag="psum", name="warm_ps")
            for w in range(8):
                nc.tensor.matmul(
                    warm_ps[:], warm_src[:, :P], warm_src[:],
                    start=(w == 0), stop=(w == 7),
                )

            # biases are tiny; land them first
            br = bp.tile([P, NT], F32, tag="br", name="br")
            nc.sync.dma_start(br[:], brd[:, :])
            bu = bp.tile([P, NT], F32, tag="bu", name="bu")
            nc.scalar.dma_start(bu[:], bud[:, :])
            bc = bp.tile([P, NT], F32, tag="bc", name="bc")
            nc.scalar.dma_start(bc[:], bcd[:, :])

            # ---- startup: first TWO r col-pairs (t=0..3) block-interleaved
            # over 8 PSUM banks: each freshly-landed operand pair feeds 8
            # matmuls (~1.7us of PE work per 384KB of DMA), so the PE
            # streams continuously while the input set loads. x8[0] rides
            # the sync HWDGE ring, split into two slot descriptors, issued
            # first: SWDGE (gpsimd) DMA completion is NOT properly waited
            # on by PE consumers (observed first-run race -> NaN).
            nc.sync.dma_start(x8t[0][:, 0:1, :], x8d[0, :, 0:1, :])
            nc.sync.dma_start(x8t[0][:, 1:2, :], x8d[0, :, 1:2, :])

            # tail-graduated chunks: small bites near the end so any DMA
            # late-arrival stalls the PE in <3.4us pieces (no HAM re-dip)
            CH = [1, 3, 4, 4, 2, 1, 1]
            CB = [0, 1, 4, 8, 12, 14, 15, 16]
            wrf = [[[None] * len(CH) for _ in range(2)] for _ in range(2)]
            ws_pre = {}  # pre-streamed steady slabs for col-pairs 2,3
            rr = 0  # round-robin ring selector
            for c, cw in enumerate(CH):
                for tp in range(2):
                    for ti in range(2):
                        t8 = wp.tile([P, 2 * cw, P], F8,
                                     tag=f"wrf{tp}{ti}_{c}",
                                     name=f"wrf{tp}{ti}_{c}", bufs=1)
                        (nc.sync if rr % 2 == 0 else nc.scalar).dma_start(
                            t8[:], wr8[tp, :, ti * KSUB + 2 * CB[c]:
                                        ti * KSUB + 2 * CB[c + 1], :]
                        )
                        rr += 1
                        wrf[tp][ti][c] = t8
                for jp in range(CB[c], CB[c + 1]):
                    if jp == 0:
                        continue  # x8[0] already issued above
                    dst = x8t[jp] if jp < 8 else h8t[jp - 8]
                    src = x8d[jp, :, :, :] if jp < 8 else h8d[jp - 8, :, :, :]
                    (nc.sync if jp % 2 == 0 else nc.scalar).dma_start(
                        dst[:], src)
                # h16 group 0 feeds the startup epilogues (~30us) -- early
                # and split across both rings so it lands before the first
                # epilogue gates col-pair 2's PSUM banks.
                if c == 1:
                    nc.sync.dma_start(h16g[0][:, :2, :], h16d[0, :, :2, :])
                    nc.scalar.dma_start(h16g[0][:, 2:, :], h16d[0, :, 2:, :])

            # col-pairs 2/3's slabs + h16 group 1 right AFTER the critical
            # chunk stream: their descriptors would otherwise share DMA
            # bandwidth with the startup set and stall the chunk matmuls.
            for tpp in (2, 3):
                ws = wp.tile([P, 2 * KSUB, P], F8, tag="wr8",
                             name=f"wr8_{tpp}", bufs=2)
                (nc.sync if tpp % 2 == 0 else nc.scalar).dma_start(
                    ws[:], wr8[tpp, :, :, :])
                ws_pre[tpp] = ws

            def r_src(jp):
                return x8t[jp] if jp < 8 else h8t[jp - 8]

            def act_r(t, m, ps):
                """rh[t] = sigmoid(psum/S + b) * h16[t], stored fp8."""
                ms = slice(m * M_CHUNK, (m + 1) * M_CHUNK)
                rt = sp.tile([P, M_CHUNK], F16, tag="rtmp", name=f"r{t}_{m}")
                nc.scalar.activation(rt[:], ps[:], SIG, bias=br[:, t:t + 1],
                                     scale=INV_S)
                nc.vector.tensor_mul(
                    rh8t[t // 2][:, t % 2, ms], rt[:], h16_ap(t, ms)
                )

            # 7 groups interleave over the arriving chunks; the 8th (t3,m1)
            # runs AFTER the chunk loop as a bridge: ~3.5us of PE work on
            # fully-resident operands that covers the window where the
            # first epilogues drain PSUM banks for col-pair 2 (otherwise
            # the PE idles >3.4us there and HAM rethrottles).
            t0_groups = [(t, m) for t in range(4) for m in range(MC)][:-1]
            pss0 = [pp.tile([P, M_CHUNK], F32, tag="psum", name=f"psg0_{i}")
                    for i in range(7)]
            for c in range(len(CH)):
                for i, (t, m) in enumerate(t0_groups):
                    ms = slice(m * M_CHUNK, (m + 1) * M_CHUNK)
                    for jp in range(CB[c], CB[c + 1]):
                        wch = wrf[t // 2][t % 2][c]
                        jj = jp - CB[c]
                        nc.tensor.matmul(
                            pss0[i][:],
                            wch[:, 2 * jj:2 * jj + 2, :],
                            r_src(jp)[:, :, ms],
                            start=(jp == 0), stop=(jp == 15),
                            perf_mode=DR,
                        )

            # h16 group 1 feeds col-pair 2/3's epilogues (~50us out)
            nc.scalar.dma_start(h16g[1][:], h16d[1, :, :, :])

            ps_br = pp.tile([P, M_CHUNK], F32, tag="psum", name="ps_br")
            ms_br = slice(1 * M_CHUNK, 2 * M_CHUNK)
            for jp in range(16):
                c = next(ci for ci in range(len(CH))
                         if CB[ci] <= jp < CB[ci + 1])
                jj = jp - CB[c]
                nc.tensor.matmul(
                    ps_br[:],
                    wrf[1][1][c][:, 2 * jj:2 * jj + 2, :],
                    r_src(jp)[:, :, ms_br],
                    start=(jp == 0), stop=(jp == 15),
                    perf_mode=DR,
                )

            for i, (t, m) in enumerate(t0_groups):
                act_r(t, m, pss0[i])
            act_r(3, 1, ps_br)

            # ---- r steady state: col-pairs 2..7, fully fp8 DoubleRow ----
            def gemm_fp8(ws, base, src_fn, npairs, psl, first, last):
                """m-interleaved DoubleRow accumulation over npairs pairs."""
                for jp in range(npairs):
                    for m in range(MC):
                        ms = slice(m * M_CHUNK, (m + 1) * M_CHUNK)
                        nc.tensor.matmul(
                            psl[m][:],
                            ws[:, base + 2 * jp:base + 2 * jp + 2, :],
                            src_fn(jp)[:, :, ms],
                            start=(first and jp == 0),
                            stop=(last and jp == npairs - 1),
                            perf_mode=DR,
                        )

            for tp in range(2, 8):
                if tp in ws_pre:
                    ws = ws_pre[tp]
                else:
                    ws = wp.tile([P, 2 * KSUB, P], F8, tag="wr8",
                                 name=f"wr8_{tp}", bufs=2)
                    (nc.sync if tp % 2 == 0 else nc.scalar).dma_start(
                        ws[:], wr8[tp, :, :, :])
                # remaining bulk fp16 groups, paced behind the slab stream
                if tp == 2:
                    nc.sync.dma_start(h16g[2][:], h16d[2, :, :, :])
                elif tp == 3:
                    nc.scalar.dma_start(h16g[3][:], h16d[3, :, :, :])
                elif tp == 4:
                    nc.sync.dma_start(x16g[2][:], x16d[2, :, :, :])
                elif tp == 5:
                    nc.scalar.dma_start(x16g[3][:], x16d[3, :, :, :])
                elif tp == 6:
                    nc.sync.dma_start(x16g[0][:], x16d[0, :, :, :])
                elif tp == 7:
                    nc.scalar.dma_start(x16g[1][:], x16d[1, :, :, :])
                for ti in range(2):
                    t = 2 * tp + ti
                    psl = [pp.tile([P, M_CHUNK], F32, tag="psum",
                                   name=f"psr{t}_{m}") for m in range(MC)]
                    gemm_fp8(ws, ti * KSUB, r_src, 16, psl, True, True)
                    for m in range(MC):
                        act_r(t, m, psl[m])

            def gemm_fp16(ws, wbase, src_ap, nsub, psl, first, last):
                for j in range(nsub):
                    for m in range(MC):
                        ms = slice(m * M_CHUNK, (m + 1) * M_CHUNK)
                        nc.tensor.matmul(
                            psl[m][:],
                            ws[:, wbase + j:wbase + j + 1, :],
                            src_ap(j, ms),
                            start=(first and j == 0),
                            stop=(last and j == nsub - 1),
                        )

            def act_u(t, m, ps):
                ms = slice(m * M_CHUNK, (m + 1) * M_CHUNK)
                nc.scalar.activation(u16t[t][:, ms], ps[:], SIG,
                                     bias=bu[:, t:t + 1], scale=INV_S)

            wu8_cur = None
            wu16_cur = None
            for t in range(NT):
                if t % 2 == 0:
                    if LEN8_U > 0:
                        wu8_cur = wp.tile([P, 2 * LEN8_U, P], F8, tag="wu8",
                                          name=f"wu8_{t // 2}", bufs=2)
                        nc.sync.dma_start(wu8_cur[:], wu8[t // 2, :, :, :])
                    wu16_cur = wp.tile([P, 2 * LEN16_U, P], F16, tag="wu16",
                                       name=f"wu16_{t // 2}", bufs=2)
                    nc.scalar.dma_start(wu16_cur[:], wu16[t // 2, :, :, :])
                if t < 8:
                    u16t[t] = wp.tile(
                        [P, B_LOC], F16, bufs=1, name=f"u{t}",
                        tag=f"wrf{['00', '01', '10', '11'][t % 4]}_{2 + t // 4}",
                    )
                else:
                    u16t[t] = res.tile([P, B_LOC], F16, tag=f"h8{t - 8}",
                                       name=f"u{t}")
                base16 = (t % 2) * LEN16_U
                psl = [pp.tile([P, M_CHUNK], F32, tag="psum",
                               name=f"psu{t}_{m}") for m in range(MC)]
                if LEN8_U > 0:
                    gemm_fp8(wu8_cur, (t % 2) * LEN8_U,
                             lambda jp: x8t[jp], N8_UX, psl, True, False)
                # fp16 remainder: x-rest subtiles then the full h half
                gemm_fp16(wu16_cur, base16,
                          lambda j, ms: x16_ap(LEN8_U + j, ms),
                          16 - LEN8_U, psl, LEN8_U == 0, False)
                gemm_fp16(wu16_cur, base16 + 16 - LEN8_U, h16_ap, 16, psl,
                          False, True)
                for m in range(MC):
                    act_u(t, m, psl[m])

            # ---- candidate: fp16 x-half + fp8 DoubleRow rh-half ----
            def cand_epilogue(t, m, mw, ps):
                ms = slice(m * mw, (m + 1) * mw)
                cand = sp.tile([P, mw], F16, tag="cand", name=f"c{t}_{m}")
                nc.scalar.activation(cand[:], ps[:], TANH,
                                     bias=bc[:, t:t + 1], scale=INV_S)
                d = sp.tile([P, mw], F16, tag="d", name=f"d{t}_{m}")
                nc.vector.tensor_sub(d[:], h16_ap(t, ms), cand[:])
                nc.vector.tensor_mul(d[:], u16t[t][:, ms], d[:])
                nc.vector.tensor_add(d[:], d[:], cand[:])
                nc.sync.dma_start(out[t, :, ms], d[:])

            wc8_cur = None
            wc16_cur = None
            for t in range(NT):
                if t % 2 == 0:
                    wc16_cur = wp.tile([P, 2 * 16, P], F16, tag="wc16",
                                       name=f"wc16_{t // 2}", bufs=2)
                    nc.scalar.dma_start(wc16_cur[:], wc16[t // 2, :, :, :])
                    wc8_cur = wp.tile([P, 2 * 16, P], F8, tag="wc8",
                                      name=f"wc8_{t // 2}", bufs=2)
                    nc.sync.dma_start(wc8_cur[:], wc8[t // 2, :, :, :])
                if t < NT - 1:
                    psl = [pp.tile([P, M_CHUNK], F32, tag="psum",
                                   name=f"psc{t}_{m}") for m in range(MC)]
                    gemm_fp16(wc16_cur, (t % 2) * 16, x16_ap, 16, psl,
                              True, False)
                    gemm_fp8(wc8_cur, (t % 2) * 16,
                             lambda jp: rh8t[jp], 8, psl, False, True)
                    for m in range(MC):
                        cand_epilogue(t, m, M_CHUNK, psl[m])
                else:
                    # taper the last col-tile: narrow sequential chunks so
                    # the post-final-matmul tail stays short
                    mw = M_CHUNK // 2
                    for m in range(B_LOC // mw):
                        ms = slice(m * mw, (m + 1) * mw)
                        ps = pp.tile([P, mw], F32, tag="psum",
                                     name=f"psct_{m}")
                        for j in range(16):
                            nc.tensor.matmul(
                                ps[:], wc16_cur[:, 16 + j:16 + j + 1, :],
                                x16_ap(j, ms),
                                start=(j == 0), stop=False,
                            )
                        for jp in range(8):
                            nc.tensor.matmul(
                                ps[:],
                                wc8_cur[:, 16 + 2 * jp:16 + 2 * jp + 2, :],
                                rh8t[jp][:, :, ms],
                                start=False, stop=(jp == 7),
                                perf_mode=DR,
                            )
                        cand_epilogue(t, m, mw, ps)

    nc.compile()
    return nc


def _get_nc():
    global _CACHED_NC
    if _CACHED_NC is None:
        _CACHED_NC = _build()
    return _CACHED_NC


def _pack_pair(w, subtiles, np_dt):
    """[4096, 2048] f32 -> [8, 128, 2*len, 128] col-pair slabs.

    slab[tp, p, ti*len + i, c] = S * w[subtiles[i]*128 + p, (2tp+ti)*128+c]
    """
    A = (w * S).reshape(KSUB, P, NT, P)[list(subtiles)]  # [len, p, t, c]
    A = A.transpose(2, 1, 0, 3)  # [t, p, len, c]
    n = len(subtiles)
    A = A.reshape(8, 2, P, n, P).transpose(0, 2, 1, 3, 4)
    return np.ascontiguousarray(A.reshape(8, P, 2 * n, P)).astype(np_dt)


def _pack_mov8(xT):
    """[2048, 1024] -> [8, 128, 2, 1024] e4m3 DoubleRow pair tiles."""
    A = xT.reshape(8, 2, P, B_LOC).transpose(0, 2, 1, 3)
    return np.ascontiguousarray(A).astype(NP_F8)


def kernel(x_t, h_tm1, input_weight, hidden_state_weight, bias):
    x_t = np.asarray(x_t, dtype=np.float32)
    h_tm1 = np.asarray(h_tm1, dtype=np.float32)
    input_weight = np.asarray(input_weight, dtype=np.float32)
    hidden_state_weight = np.asarray(hidden_state_weight, dtype=np.float32)
    bias = np.asarray(bias, dtype=np.float32)

    u = UNITS
    # per-gate stacked weights [x; h] -> [4096, 2048] each
    w_r = np.concatenate([input_weight[:, :u], hidden_state_weight[:, :u]], 0)
    w_u = np.concatenate(
        [input_weight[:, u:2 * u], hidden_state_weight[:, u:2 * u]], 0)
    w_c = np.concatenate(
        [input_weight[:, 2 * u:], hidden_state_weight[:, 2 * u:]], 0)

    wr8_np = _pack_pair(w_r, range(32), NP_F8)
    wu8_np = _pack_pair(w_u, range(LEN8_U), NP_F8)
    wu16_np = _pack_pair(w_u, list(range(LEN8_U, 16)) + list(range(16, 32)),
                         NP_F16)
    wc16_np = _pack_pair(w_c, range(16), NP_F16)
    wc8_np = _pack_pair(w_c, range(16, 32), NP_F8)
    br_np = np.ascontiguousarray(bias[:u].reshape(NT, P).T, dtype=np.float32)
    bu_np = np.ascontiguousarray(bias[u:2 * u].reshape(NT, P).T,
                                 dtype=np.float32)
    bc_np = np.ascontiguousarray(bias[2 * u:].reshape(NT, P).T,
                                 dtype=np.float32)

    in_maps = []
    for i in range(N_CORES):
        sl = slice(i * B_LOC, (i + 1) * B_LOC)
        xT = np.ascontiguousarray(x_t[sl].T)   # [2048, 1024]
        hT = np.ascontiguousarray(h_tm1[sl].T)
        in_maps.append({
            "x8d": _pack_mov8(xT),
            "h8d": _pack_mov8(hT),
            "x16d": np.ascontiguousarray(
                xT.reshape(4, 4, P, B_LOC).transpose(0, 2, 1, 3)
            ).astype(NP_F16),
            "h16d": np.ascontiguousarray(
                hT.reshape(4, 4, P, B_LOC).transpose(0, 2, 1, 3)
            ).astype(NP_F16),
            "wr8": wr8_np, "wu8": wu8_np, "wu16": wu16_np,
            "wc16": wc16_np, "wc8": wc8_np,
            "brd": br_np, "bud": bu_np, "bcd": bc_np,
        })

    nc = _get_nc()
    res = run_bass_kernel_spmd(
        nc, in_maps, core_ids=list(range(N_CORES)), trace=TRACE
    )
    global LAST_RESULTS
    LAST_RESULTS = res

    h_t = np.empty((BATCH, UNITS), dtype=np.float32)
    for i in range(N_CORES):
        o = np.asarray(res.results[i]["out"]).astype(np.float32)
        h_t[i * B_LOC:(i + 1) * B_LOC] = o.reshape(UNITS, B_LOC).T
    return h_t


# revision 40
# speedup vs baseline: 1.0153x; 1.0153x over previous
"""GRU cell (AnotherGRUCell) on 8 TRN2 NeuronCores.

Strategy: pure data-parallel over batch (8192 rows -> 1024 rows/core),
weights replicated. No collectives.

All on-chip compute is in TRANSPOSED layout (units on the partition
axis, batch on the free axis), as in the bf16 baseline, but with a
mixed fp8/fp16 precision scheme chosen from a host-side error model
(validated against HW to 4 digits on the bf16 version):

  - matmul halves run either as fp8e4 (e4m3) DoubleRow matmuls (2
    contraction rows per PE cell per cycle -> ~2x bf16 throughput) or
    as fp16 matmuls (same speed as bf16 but 10-bit mantissa, which
    removes nearly all of the bf16 quantization error).
  - error budget (rel tol 2e-2): per-half err^2 contributions measured
    on the real inputs: r-gate halves ~0.15e-4 each, u-gate halves
    ~1.63e-4, cand-x 4.04e-4, cand-rh 1.29e-4. Chosen config: r fully
    fp8, cand rh-half fp8, u x-half fully fp8 (N8_UX=8 pairs),
    everything else fp16 -> rel err 1.792e-2 (sim == HW to 6 digits).
  - all weights (both dtypes) are pre-scaled by S=32 so fp8 weights
    avoid denormals; the sigmoid/tanh activation reads PSUM with
    scale=1/S, folding the rescale into the existing ScalarE op.

Layouts: fp8 moving operands are packed as [128, 2, B] pair tiles
(DoubleRow wants 3D APs [K=128, Ko=2, N] over two adjacent 128-row
k-subtiles); fp8 weights as [128, 2*len8, 128] col-pair slabs; fp16
weights as [128, len16, 128] per-col-tile slabs.
"""

import numpy as np
import ml_dtypes

import concourse.bacc as bacc
import concourse.tile as tile
import concourse.mybir as mybir
from concourse.bass_utils import run_bass_kernel_spmd

N_CORES = 8
UNITS = 2048
IN_DIM = 2048
BATCH = 8192
B_LOC = BATCH // N_CORES  # 1024 batch rows per core

P = 128
KSUB = 32            # 32 contraction k-subtiles for [x; h] (16 x + 16 h)
NT = UNITS // P      # 16 col-tiles per gate (r / u / cand)
M_CHUNK = 512
MC = B_LOC // M_CHUNK  # 2 moving chunks per core

S = 32.0             # weight pre-scale (fp8 denormal avoidance)

# fp8 coverage (in DoubleRow k-pairs, each pair = 2 k-subtiles of 128):
# r: both halves fully fp8; u: full x-half + the LAST h pair (subtiles
# 30/31 = h8 pair 7, chosen so the u16-on-h8 buffer aliasing stays
# legal); c: rh-half.
N8_UX = 8
N8_UH = 1
LEN8_U = 2 * (N8_UX + N8_UH)  # fp8 k-subtiles per u col-tile
LEN16_U = 32 - LEN8_U         # fp16 k-subtiles per u col-tile

F8 = mybir.dt.float8e4
F16 = mybir.dt.float16
F32 = mybir.dt.float32
BF16 = mybir.dt.bfloat16
NP_F8 = ml_dtypes.float8_e4m3
NP_F16 = np.float16
DR = mybir.MatmulPerfMode.DoubleRow

_CACHED_NC = None

# test.py sets TRACE=True to capture the NTFF profile (exec_time_ns +
# perfetto trace); the graded path leaves it off. LAST_RESULTS holds the
# BassKernelResults of the most recent run.
TRACE = False
LAST_RESULTS = None


def _build():
    nc = bacc.Bacc("TRN2", target_bir_lowering=False, debug=False)

    x8d = nc.dram_tensor("x8d", [8, P, 2, B_LOC], F8, kind="ExternalInput")
    h8d = nc.dram_tensor("h8d", [8, P, 2, B_LOC], F8, kind="ExternalInput")
    # fp16 moving tensors in groups of 4 k-subtiles (fewer tiles -> fewer
    # PE first-use semaphore beats)
    x16d = nc.dram_tensor("x16d", [4, P, 4, B_LOC], F16, kind="ExternalInput")
    h16d = nc.dram_tensor("h16d", [4, P, 4, B_LOC], F16, kind="ExternalInput")
    # weights: fp8 slabs per col-PAIR [pair, 128, 2*len8, 128]; fp16
    # slabs per col-tile [t, 128, len16, 128]
    wr8 = nc.dram_tensor("wr8", [8, P, 2 * KSUB, P], F8, kind="ExternalInput")
    wu8 = nc.dram_tensor("wu8", [8, P, 2 * LEN8_U, P], F8, kind="ExternalInput")
    wu16 = nc.dram_tensor("wu16", [8, P, 2 * LEN16_U, P], F16,
                          kind="ExternalInput")
    wc16 = nc.dram_tensor("wc16", [8, P, 2 * 16, P], F16, kind="ExternalInput")
    wc8 = nc.dram_tensor("wc8", [8, P, 2 * 16, P], F8, kind="ExternalInput")
    brd = nc.dram_tensor("brd", [P, NT], F32, kind="ExternalInput")
    bud = nc.dram_tensor("bud", [P, NT], F32, kind="ExternalInput")
    bcd = nc.dram_tensor("bcd", [P, NT], F32, kind="ExternalInput")
    out = nc.dram_tensor("out", [NT, P, B_LOC], F16, kind="ExternalOutput")

    SIG = mybir.ActivationFunctionType.Sigmoid
    TANH = mybir.ActivationFunctionType.Tanh
    INV_S = 1.0 / S

    with tile.TileContext(nc) as tc:
        with (
            tc.tile_pool(name="resident", bufs=1) as res,
            tc.tile_pool(name="wslab", bufs=2) as wp,
            tc.tile_pool(name="psum", bufs=8, space="PSUM") as pp,
            tc.tile_pool(name="stage", bufs=2) as sp,
            tc.tile_pool(name="bias", bufs=1) as bp,
        ):
            x8t = [res.tile([P, 2, B_LOC], F8, tag=f"x8{j}", name=f"x8{j}")
                   for j in range(8)]
            h8t = [res.tile([P, 2, B_LOC], F8, tag=f"h8{j}", name=f"h8{j}")
                   for j in range(8)]
            x16g = [res.tile([P, 4, B_LOC], F16, tag=f"x16g{g}",
                             name=f"x16g{g}") for g in range(4)]
            h16g = [res.tile([P, 4, B_LOC], F16, tag=f"h16g{g}",
                             name=f"h16g{g}") for g in range(4)]
            rh8t = [res.tile([P, 2, B_LOC], F8, tag=f"rh{j}", name=f"rh{j}")
                    for j in range(8)]
            # u16 tiles are all aliased onto buffers that go dead before the
            # u phase: [0..7] onto the largest startup weight-chunk tiles,
            # [8..15] onto the h8 pair tiles (same 2KB/partition footprint);
            # the tile framework sequences the reuse. Allocated lazily in
            # the u loop.
            u16t = [None] * 16

            def x16_ap(j, ms):
                return x16g[j // 4][:, j % 4, ms]

            def h16_ap(j, ms):
                return h16g[j // 4][:, j % 4, ms]

            # PE warm-up vs the HAM clock gate (see bf16 baseline notes):
            # 8 dummy matmuls sized to end before real operands land.
            warm_src = sp.tile([P, M_CHUNK], BF16, tag="rtmp",
                               name="warm_src")
            nc.gpsimd.memset(warm_src[:], 0.0)
            warm_ps = pp.tile([P, M_CHUNK], F32, tag="psum", name="warm_ps")
            for w in range(8):
                nc.tensor.matmul(
                    warm_ps[:], warm_src[:, :P], warm_src[:],
                    start=(w == 0), stop=(w == 7),
                )

            # biases are tiny; land them first
            br = bp.tile([P, NT], F32, tag="br", name="br")
            nc.sync.dma_start(br[:], brd[:, :])
            bu = bp.tile([P, NT], F32, tag="bu", name="bu")
            nc.scalar.dma_start(bu[:], bud[:, :])
            bc = bp.tile([P, NT], F32, tag="bc", name="bc")
            nc.scalar.dma_start(bc[:], bcd[:, :])

            # ---- startup: first TWO r col-pairs (t=0..3) block-interleaved
            # over 8 PSUM banks: each freshly-landed operand pair feeds 8
            # matmuls (~1.7us of PE work per 384KB of DMA), so the PE
            # streams continuously while the input set loads. x8[0] rides
            # the sync HWDGE ring, split into two slot descriptors, issued
            # first: SWDGE (gpsimd) DMA completion is NOT properly waited
            # on by PE consumers (observed first-run race -> NaN).
            nc.sync.dma_start(x8t[0][:, 0:1, :], x8d[0, :, 0:1, :])
            nc.sync.dma_start(x8t[0][:, 1:2, :], x8d[0, :, 1:2, :])

            # tail-graduated chunks: small bites near the end so any DMA
            # late-arrival stalls the PE in <3.4us pieces (no HAM re-dip)
            CH = [1, 3, 4, 4, 2, 1, 1]
            CB = [0, 1, 4, 8, 12, 14, 15, 16]
            wrf = [[[None] * len(CH) for _ in range(2)] for _ in range(2)]
            ws_pre = {}  # pre-streamed steady slabs for col-pairs 2,3
            rr = 0  # round-robin ring selector
            for c, cw in enumerate(CH):
                for tp in range(2):
                    for ti in range(2):
                        t8 = wp.tile([P, 2 * cw, P], F8,
                                     tag=f"wrf{tp}{ti}_{c}",
                                     name=f"wrf{tp}{ti}_{c}", bufs=1)
                        (nc.sync if rr % 2 == 0 else nc.scalar).dma_start(
                            t8[:], wr8[tp, :, ti * KSUB + 2 * CB[c]:
                                        ti * KSUB + 2 * CB[c + 1], :]
                        )
                        rr += 1
                        wrf[tp][ti][c] = t8
                for jp in range(CB[c], CB[c + 1]):
                    if jp == 0:
                        continue  # x8[0] already issued above
                    dst = x8t[jp] if jp < 8 else h8t[jp - 8]
                    src = x8d[jp, :, :, :] if jp < 8 else h8d[jp - 8, :, :, :]
                    (nc.sync if jp % 2 == 0 else nc.scalar).dma_start(
                        dst[:], src)
                # h16 group 0 feeds the startup epilogues (~30us) -- early
                # and split across both rings so it lands before the first
                # epilogue gates col-pair 2's PSUM banks.
                if c == 1:
                    nc.sync.dma_start(h16g[0][:, :2, :], h16d[0, :, :2, :])
                    nc.scalar.dma_start(h16g[0][:, 2:, :], h16d[0, :, 2:, :])

            # col-pairs 2/3's slabs + h16 group 1 right AFTER the critical
            # chunk stream: their descriptors would otherwise share DMA
            # bandwidth with the startup set and stall the chunk matmuls.
            for tpp in (2, 3):
                ws = wp.tile([P, 2 * KSUB, P], F8, tag="wr8",
                             name=f"wr8_{tpp}", bufs=2)
                (nc.sync if tpp % 2 == 0 else nc.scalar).dma_start(
                    ws[:], wr8[tpp, :, :, :])
                ws_pre[tpp] = ws

            def r_src(jp):
                return x8t[jp] if jp < 8 else h8t[jp - 8]

            def act_r(t, m, ps):
                """rh[t] = sigmoid(psum/S + b) * h16[t], stored fp8."""
                ms = slice(m * M_CHUNK, (m + 1) * M_CHUNK)
                rt = sp.tile([P, M_CHUNK], F16, tag="rtmp", name=f"r{t}_{m}")
                nc.scalar.activation(rt[:], ps[:], SIG, bias=br[:, t:t + 1],
                                     scale=INV_S)
                nc.vector.tensor_mul(
                    rh8t[t // 2][:, t % 2, ms], rt[:], h16_ap(t, ms)
                )

            # 7 groups interleave over the arriving chunks; the 8th (t3,m1)
            # runs AFTER the chunk loop as a bridge: ~3.5us of PE work on
            # fully-resident operands that covers the window where the
            # first epilogues drain PSUM banks for col-pair 2 (otherwise
            # the PE idles >3.4us there and HAM rethrottles).
            t0_groups = [(t, m) for t in range(4) for m in range(MC)][:-1]
            pss0 = [pp.tile([P, M_CHUNK], F32, tag="psum", name=f"psg0_{i}")
                    for i in range(7)]
            for c in range(len(CH)):
                for i, (t, m) in enumerate(t0_groups):
                    ms = slice(m * M_CHUNK, (m + 1) * M_CHUNK)
                    for jp in range(CB[c], CB[c + 1]):
                        wch = wrf[t // 2][t % 2][c]
                        jj = jp - CB[c]
                        nc.tensor.matmul(
                            pss0[i][:],
                            wch[:, 2 * jj:2 * jj + 2, :],
                            r_src(jp)[:, :, ms],
                            start=(jp == 0), stop=(jp == 15),
                            perf_mode=DR,
                        )

            # h16 group 1 feeds col-pair 2/3's epilogues (~50us out)
            nc.scalar.dma_start(h16g[1][:], h16d[1, :, :, :])

            ps_br = pp.tile([P, M_CHUNK], F32, tag="psum", name="ps_br")
            ms_br = slice(1 * M_CHUNK, 2 * M_CHUNK)
            for jp in range(16):
                c = next(ci for ci in range(len(CH))
                         if CB[ci] <= jp < CB[ci + 1])
                jj = jp - CB[c]
                nc.tensor.matmul(
                    ps_br[:],
                    wrf[1][1][c][:, 2 * jj:2 * jj + 2, :],
                    r_src(jp)[:, :, ms_br],
                    start=(jp == 0), stop=(jp == 15),
                    perf_mode=DR,
                )

            for i, (t, m) in enumerate(t0_groups):
                act_r(t, m, pss0[i])
            act_r(3, 1, ps_br)

            # ---- r steady state: col-pairs 2..7, fully fp8 DoubleRow ----
            def gemm_fp8(ws, base, src_fn, npairs, psl, first, last):
                """m-interleaved DoubleRow accumulation over npairs pairs."""
                for jp in range(npairs):
                    for m in range(MC):
                        ms = slice(m * M_CHUNK, (m + 1) * M_CHUNK)
                        nc.tensor.matmul(
                            psl[m][:],
                            ws[:, base + 2 * jp:base + 2 * jp + 2, :],
                            src_fn(jp)[:, :, ms],
                            start=(first and jp == 0),
                            stop=(last and jp == npairs - 1),
                            perf_mode=DR,
                        )

            for tp in range(2, 8):
                if tp in ws_pre:
                    ws = ws_pre[tp]
                else:
                    ws = wp.tile([P, 2 * KSUB, P], F8, tag="wr8",
                                 name=f"wr8_{tp}", bufs=2)
                    (nc.sync if tp % 2 == 0 else nc.scalar).dma_start(
                        ws[:], wr8[tp, :, :, :])
                # remaining bulk fp16 groups, paced behind the slab stream
                if tp == 2:
                    nc.sync.dma_start(h16g[2][:], h16d[2, :, :, :])
                elif tp == 3:
                    nc.scalar.dma_start(h16g[3][:], h16d[3, :, :, :])
                elif tp == 4:
                    nc.sync.dma_start(x16g[2][:], x16d[2, :, :, :])
                elif tp == 5:
                    nc.scalar.dma_start(x16g[3][:], x16d[3, :, :, :])
                elif tp == 6:
                    nc.sync.dma_start(x16g[0][:], x16d[0, :, :, :])
                elif tp == 7:
                    nc.scalar.dma_start(x16g[1][:], x16d[1, :, :, :])
                for ti in range(2):
                    t = 2 * tp + ti
                    psl = [pp.tile([P, M_CHUNK], F32, tag="psum",
                                   name=f"psr{t}_{m}") for m in range(MC)]
                    gemm_fp8(ws, ti * KSUB, r_src, 16, psl, True, True)
                    for m in range(MC):
                        act_r(t, m, psl[m])

            def gemm_fp16(ws, wbase, src_ap, nsub, psl, first, last):
                for j in range(nsub):
                    for m in range(MC):
                        ms = slice(m * M_CHUNK, (m + 1) * M_CHUNK)
                        nc.tensor.matmul(
                            psl[m][:],
                            ws[:, wbase + j:wbase + j + 1, :],
                            src_ap(j, ms),
                            start=(first and j == 0),
                            stop=(last and j == nsub - 1),
                        )

            def act_u(t, m, ps):
                ms = slice(m * M_CHUNK, (m + 1) * M_CHUNK)
                nc.scalar.activation(u16t[t][:, ms], ps[:], SIG,
                                     bias=bu[:, t:t + 1], scale=INV_S)

            wu8_cur = None
            wu16_cur = None
            for t in range(NT):
                if t % 2 == 0:
                    if LEN8_U > 0:
                        wu8_cur = wp.tile([P, 2 * LEN8_U, P], F8, tag="wu8",
                                          name=f"wu8_{t // 2}", bufs=2)
                        nc.sync.dma_start(wu8_cur[:], wu8[t // 2, :, :, :])
                    wu16_cur = wp.tile([P, 2 * LEN16_U, P], F16, tag="wu16",
                                       name=f"wu16_{t // 2}", bufs=2)
                    nc.scalar.dma_start(wu16_cur[:], wu16[t // 2, :, :, :])
                if t < 8:
                    u16t[t] = wp.tile(
                        [P, B_LOC], F16, bufs=1, name=f"u{t}",
                        tag=f"wrf{['00', '01', '10', '11'][t % 4]}_{2 + t // 4}",
                    )
                else:
                    u16t[t] = res.tile([P, B_LOC], F16, tag=f"h8{t - 8}",
                                       name=f"u{t}")
                base16 = (t % 2) * LEN16_U
                psl = [pp.tile([P, M_CHUNK], F32, tag="psum",
                               name=f"psu{t}_{m}") for m in range(MC)]
                gemm_fp8(wu8_cur, (t % 2) * LEN8_U,
                         lambda jp: x8t[jp] if jp < 8 else h8t[7],
                         N8_UX + N8_UH, psl, True, False)
                # fp16 remainder: h subtiles 16..29 (30/31 ride fp8 above)
                gemm_fp16(wu16_cur, base16, h16_ap, LEN16_U, psl,
                          False, True)
                for m in range(MC):
                    act_u(t, m, psl[m])

            # ---- candidate: fp16 x-half + fp8 DoubleRow rh-half ----
            def cand_epilogue(t, m, mw, ps):
                ms = slice(m * mw, (m + 1) * mw)
                cand = sp.tile([P, mw], F16, tag="cand", name=f"c{t}_{m}")
                nc.scalar.activation(cand[:], ps[:], TANH,
                                     bias=bc[:, t:t + 1], scale=INV_S)
                d = sp.tile([P, mw], F16, tag="d", name=f"d{t}_{m}")
                nc.vector.tensor_sub(d[:], h16_ap(t, ms), cand[:])
                nc.vector.tensor_mul(d[:], u16t[t][:, ms], d[:])
                nc.vector.tensor_add(d[:], d[:], cand[:])
                nc.sync.dma_start(out[t, :, ms], d[:])

            wc8_cur = None
            wc16_cur = None
            for t in range(NT):
                if t % 2 == 0:
                    wc16_cur = wp.tile([P, 2 * 16, P], F16, tag="wc16",
                                       name=f"wc16_{t // 2}", bufs=2)
                    nc.scalar.dma_start(wc16_cur[:], wc16[t // 2, :, :, :])
                    wc8_cur = wp.tile([P, 2 * 16, P], F8, tag="wc8",
                                      name=f"wc8_{t // 2}", bufs=2)
                    nc.sync.dma_start(wc8_cur[:], wc8[t // 2, :, :, :])
                if t < NT - 1:
                    psl = [pp.tile([P, M_CHUNK], F32, tag="psum",
                                   name=f"psc{t}_{m}") for m in range(MC)]
                    gemm_fp16(wc16_cur, (t % 2) * 16, x16_ap, 16, psl,
                              True, False)
                    gemm_fp8(wc8_cur, (t % 2) * 16,
                             lambda jp: rh8t[jp], 8, psl, False, True)
                    for m in range(MC):
                        cand_epilogue(t, m, M_CHUNK, psl[m])
                else:
                    # taper the last col-tile: narrow sequential chunks so
                    # the post-final-matmul tail stays short
                    mw = M_CHUNK // 2
                    for m in range(B_LOC // mw):
                        ms = slice(m * mw, (m + 1) * mw)
                        ps = pp.tile([P, mw], F32, tag="psum",
                                     name=f"psct_{m}")
                        for j in range(16):
                            nc.tensor.matmul(
                                ps[:], wc16_cur[:, 16 + j:16 + j + 1, :],
                                x16_ap(j, ms),
                                start=(j == 0), stop=False,
                            )
                        for jp in range(8):
                            nc.tensor.matmul(
                                ps[:],
                                wc8_cur[:, 16 + 2 * jp:16 + 2 * jp + 2, :],
                                rh8t[jp][:, :, ms],
                                start=False, stop=(jp == 7),
                                perf_mode=DR,
                            )
                        cand_epilogue(t, m, mw, ps)

    nc.compile()
    return nc


def _get_nc():
    global _CACHED_NC
    if _CACHED_NC is None:
        _CACHED_NC = _build()
    return _CACHED_NC


def _pack_pair(w, subtiles, np_dt):
    """[4096, 2048] f32 -> [8, 128, 2*len, 128] col-pair slabs.

    slab[tp, p, ti*len + i, c] = S * w[subtiles[i]*128 + p, (2tp+ti)*128+c]
    """
    A = (w * S).reshape(KSUB, P, NT, P)[list(subtiles)]  # [len, p, t, c]
    A = A.transpose(2, 1, 0, 3)  # [t, p, len, c]
    n = len(subtiles)
    A = A.reshape(8, 2, P, n, P).transpose(0, 2, 1, 3, 4)
    return np.ascontiguousarray(A.reshape(8, P, 2 * n, P)).astype(np_dt)


def _pack_mov8(xT):
    """[2048, 1024] -> [8, 128, 2, 1024] e4m3 DoubleRow pair tiles."""
    A = xT.reshape(8, 2, P, B_LOC).transpose(0, 2, 1, 3)
    return np.ascontiguousarray(A).astype(NP_F8)


def kernel(x_t, h_tm1, input_weight, hidden_state_weight, bias):
    x_t = np.asarray(x_t, dtype=np.float32)
    h_tm1 = np.asarray(h_tm1, dtype=np.float32)
    input_weight = np.asarray(input_weight, dtype=np.float32)
    hidden_state_weight = np.asarray(hidden_state_weight, dtype=np.float32)
    bias = np.asarray(bias, dtype=np.float32)

    u = UNITS
    # per-gate stacked weights [x; h] -> [4096, 2048] each
    w_r = np.concatenate([input_weight[:, :u], hidden_state_weight[:, :u]], 0)
    w_u = np.concatenate(
        [input_weight[:, u:2 * u], hidden_state_weight[:, u:2 * u]], 0)
    w_c = np.concatenate(
        [input_weight[:, 2 * u:], hidden_state_weight[:, 2 * u:]], 0)

    wr8_np = _pack_pair(w_r, range(32), NP_F8)
    wu8_np = _pack_pair(w_u, list(range(16)) + [30, 31], NP_F8)
    wu16_np = _pack_pair(w_u, range(16, 30), NP_F16)
    wc16_np = _pack_pair(w_c, range(16), NP_F16)
    wc8_np = _pack_pair(w_c, range(16, 32), NP_F8)
    br_np = np.ascontiguousarray(bias[:u].reshape(NT, P).T, dtype=np.float32)
    bu_np = np.ascontiguousarray(bias[u:2 * u].reshape(NT, P).T,
                                 dtype=np.float32)
    bc_np = np.ascontiguousarray(bias[2 * u:].reshape(NT, P).T,
                                 dtype=np.float32)

    in_maps = []
    for i in range(N_CORES):
        sl = slice(i * B_LOC, (i + 1) * B_LOC)
        xT = np.ascontiguousarray(x_t[sl].T)   # [2048, 1024]
        hT = np.ascontiguousarray(h_tm1[sl].T)
        in_maps.append({
            "x8d": _pack_mov8(xT),
            "h8d": _pack_mov8(hT),
            "x16d": np.ascontiguousarray(
                xT.reshape(4, 4, P, B_LOC).transpose(0, 2, 1, 3)
            ).astype(NP_F16),
            "h16d": np.ascontiguousarray(
                hT.reshape(4, 4, P, B_LOC).transpose(0, 2, 1, 3)
            ).astype(NP_F16),
            "wr8": wr8_np, "wu8": wu8_np, "wu16": wu16_np,
            "wc16": wc16_np, "wc8": wc8_np,
            "brd": br_np, "bud": bu_np, "bcd": bc_np,
        })

    nc = _get_nc()
    res = run_bass_kernel_spmd(
        nc, in_maps, core_ids=list(range(N_CORES)), trace=TRACE
    )
    global LAST_RESULTS
    LAST_RESULTS = res

    h_t = np.empty((BATCH, UNITS), dtype=np.float32)
    for i in range(N_CORES):
        o = np.asarray(res.results[i]["out"]).astype(np.float32)
        h_t[i * B_LOC:(i + 1) * B_LOC] = o.reshape(UNITS, B_LOC).T
    return h_t
